# revision 27
# baseline (speedup 1.0000x reference)
"""Trainium2 Bass kernel for nn_DistanceLoss (retrieval_knn, 5-way 5-shot).

v4: restructured fp8 DoubleRow kernel.

Computation (per reference):
    q  = relu(queries.flat @ W.T + b)          [5600, 1024]
    se = relu(support.flat @ W.T + b)          [1400, 1024]
    d2 = q_sq + s_sq - 2 q @ se.T              [5600, 1400]
    out[q, c] = -mean_t min_{j in class c} sqrt(relu(d2))

Sharding (8 cores):
  - data-parallel over queries: 13 queries (728 rows) per core (padded 100->104)
  - support projection sharded by support cols (175/core), AllGathered (fp8)
    together with an fp16 -s_sq row (replaces the old fp8 4-way split).

Key changes vs the 155us baseline:
  - W is streamed per k-pair (24 tiles) so phase A matmuls issue as W arrives
    from HBM; phase B runs (m-group, g, chunk) with 6 psum banks so its first
    pass overlaps the tail of the input stream.
  - the s_sq reduction matmul is emitted after phase B's first pass, so the
    PE never idles on the A-tail ACT/DVE chain (which used to trigger a HAM
    re-throttle to K=4/8).
  - q_sq flipped: stationary = ones column (1-col LdWeights), moving = sqt
    row chunk -> [1, nch] psum rows at ~1 col/cycle instead of N=1 matmuls
    with a 128-col weight load each; transposed to columns by a DMA bounce
    through DRAM.
  - s_sq shipped as one fp16 payload row; folded via DVE copy + broadcast.
"""

import os
import sys

if "/opt/trn_rl_repo" not in sys.path:
    sys.path.insert(0, "/opt/trn_rl_repo")

import ml_dtypes
import numpy as np

import concourse.bacc as bacc
import concourse.mybir as mybir
import concourse.tile as tile
from concourse.bass_utils import run_bass_kernel_spmd

WAY, SHOT, T = 5, 5, 56
D_IN, D_OUT = 6144, 1024
N_Q, N_S = 100, 25
N_CORES = 8
QPC = 13                 # queries per core (104 padded)
RPC = QPC * T            # 728 query rows per core
NQR = N_CORES * RPC      # 5824 padded query rows
NSR = N_S * T            # 1400 support rows
SPC = NSR // N_CORES     # 175 support cols per core
KP = D_IN // 256         # 24 k-pairs (DoubleRow: 2x128 contraction each)
RPCP = 768               # q8 row stride (64B-aligned for dual-fp8 ldweights)
CLS = NSR // WAY         # 280 columns per class
MT = (RPC + 127) // 128  # 6 row tiles (5x128 + 88)
WSCALE = 64.0            # host multiplies W by this before fp8 cast
CH0 = 384                # phase B row chunk boundary (3 row tiles)
CHUNKS = ((0, CH0), (CH0, RPC))
MGROUPS = ((0, 3), (3, 6), (6, 8))
AGROWS = 514             # payload rows: 512 seT (pair-ilv) + 1 fp16 s_sq

f32 = mybir.dt.float32
f16 = mybir.dt.float16
f8 = mybir.dt.float8e4
AF = mybir.ActivationFunctionType
ALU = mybir.AluOpType
AX = mybir.AxisListType
DR = mybir.MatmulPerfMode.DoubleRow

_MODE = os.environ.get("KERNEL_MODE", "full")


def _build_nc():
    nc = bacc.Bacc("TRN2", target_bir_lowering=False, debug=False,
                   num_devices=N_CORES)
    qT = nc.dram_tensor("qT", [6, 128, 4, RPC, 2], f8, kind="ExternalInput")
    w2 = nc.dram_tensor("w2", [KP, 128, 2, D_OUT], f8, kind="ExternalInput")
    sT = nc.dram_tensor("sT", [6, 128, 4, SPC, 2], f8, kind="ExternalInput")
    bq = nc.dram_tensor("bq", [128, 8], f32, kind="ExternalInput")
    bs = nc.dram_tensor("bs", [128, 8], f32, kind="ExternalInput")
    mmask = nc.dram_tensor("mmask", [MT * 128, QPC], f32, kind="ExternalInput")
    csts = nc.dram_tensor("csts", [128, 2], f16, kind="ExternalInput")
    out = nc.dram_tensor("out", [QPC, WAY], f32, kind="ExternalOutput")

    with tile.TileContext(nc) as tc:
        _body(tc, nc, qT, w2, sT, bq, bs, mmask, csts, out)
    nc.finalize()
    return nc


def _body(tc, nc, qT, w2, sT, bq, bs, mmask, csts, out):
    persist_ctx = tc.tile_pool(name="persist", bufs=1)
    persist = persist_ctx.__enter__()

    def ptile(shape, name, dtype=f32):
        return persist.tile(shape, dtype, tag=name, name=name)

    # ---- persistent tiles ----
    w2s = [ptile([128, 2, D_OUT], f"w2_{g}", f8) for g in range(KP)]
    sI = [ptile([128, 4, SPC, 2], f"sI{t}", f8) for t in range(6)]
    qI = [ptile([128, 4, RPC, 2], f"qI{t}", f8) for t in range(6)]
    q8 = ptile([128, 8, RPCP], "q8", f8)        # relu'd fp8 query activations
    sqt = ptile([128, 8, RPCP], "sqt", f16)     # q8^2 (exact in fp16)
    sePI = ptile([128, 4, NSR, 2], "sePI", f8)  # gathered 2*se.T, pair-ilv
    seLh = ptile([128, 4, 176, 2], "seLh", f8)  # local 2*se.T, pair-ilv
    sq4 = ptile([128, 8, SPC], "sq4", f16)      # seLh^2
    sqs = [ptile([128, SPC], f"sqs{i}", f16) for i in range(3)]
    nstA = ptile([2, 176], "nstA", f16)         # -s_sq local cols (fp16)+pad
    nsr1h = ptile([1, NSR], "nsr1h", f16)       # gathered -s_sq row (fp16)
    nsr1 = ptile([1, NSR], "nsr1")              # -s_sq row (f32)
    nsrB = ptile([128, NSR], "nsrB")            # broadcast -s_sq
    qrow = ptile([1, RPC], "qrow")              # q_sq as a row
    qsqc = ptile([128, MT], "qsqc")             # q_sq as columns per row tile
    bqc = ptile([128, 8], "bqc")
    bsc = ptile([128, 8], "bsc")
    cst = ptile([128, 2], "cst", f16)           # col0=1.0 (qsq), col1=-0.25
    mkt = [ptile([128, QPC], f"mk{mt}") for mt in range(MT)]
    mins = [ptile([128, WAY], f"mins{mt}") for mt in range(MT)]

    # ---- DMA: small constants on the gpsimd queue ----
    for mt in range(MT):
        nc.gpsimd.dma_start(out=mkt[mt][:],
                            in_=mmask[mt * 128:(mt + 1) * 128, :])
    nc.gpsimd.dma_start(out=bqc[:], in_=bq[:])
    nc.gpsimd.dma_start(out=bsc[:], in_=bs[:])
    nc.gpsimd.dma_start(out=cst[:], in_=csts[:])

    # ---- DMA: big streams on the sync queue (phase-A stream first) ----
    for g in range(KP):
        nc.sync.dma_start(out=w2s[g][:], in_=w2[g])
        if g % 4 == 0:
            nc.sync.dma_start(out=sI[g // 4][:], in_=sT[g // 4])
    for t in range(6):
        nc.sync.dma_start(out=qI[t][:], in_=qT[t])

    # ---- memsets (payload pad + psum-tail safety) ----
    nc.vector.memset(seLh[:], 0.0)
    nc.vector.memset(nstA[:], 0.0)
    nc.vector.memset(qsqc[:], 0.0)
    for mt in range(MT):
        nc.vector.memset(mins[mt][:], 0.0)

    # ---- allgather + bounce buffers ----
    dram_ctx = tc.tile_pool(name="dram", bufs=1, space="DRAM")
    dram = dram_ctx.__enter__()
    ag_in = dram.tile([AGROWS, 352], f8, tag="ag_in", name="ag_in")
    qsd = dram.tile([1, RPC], f32, tag="qsd", name="qsd")
    ag_out = dram.tile([N_CORES, AGROWS, 352], f8, tag="ag_out",
                       name="ag_out",
                       addr_space="Local" if _MODE == "nocc" else "Shared")

    # ---- phase A: support projection seT = W @ S, direct layout ----
    with tc.tile_pool(name="psA", bufs=1, space="PSUM") as psA_pool:
        psA = [psA_pool.tile([128, SPC], f32, tag=f"psA{m}", name=f"psA{m}")
               for m in range(8)]
        for g in range(KP):
            smov = sI[g // 4][:, g % 4, :, :].rearrange("p n t -> p t n")
            for ms in range(8):
                nc.tensor.matmul(
                    psA[ms][:],
                    w2s[g][:, :, ms * 128:(ms + 1) * 128],
                    smov,
                    start=(g == 0), stop=(g == KP - 1),
                    perf_mode=DR,
                )
        for ms in range(8):
            # seL = relu(2*(z + b)) = 2*relu(z+b); psum holds 64*z
            nc.scalar.activation(seLh[:, ms // 2, 0:SPC, ms % 2], psA[ms][:],
                                 AF.Relu, bias=bsc[:, ms:ms + 1],
                                 scale=2.0 / WSCALE)
        for ms in range(8):
            nc.scalar.activation(sq4[:, ms, :], seLh[:, ms // 2, 0:SPC, ms % 2],
                                 AF.Square)
        # sum of squares over the 8 dout slots (tree on DVE)
        nc.vector.tensor_tensor(sqs[0][:], sq4[:, 0, :], sq4[:, 1, :],
                                op=ALU.add)
        nc.vector.tensor_tensor(sqs[1][:], sq4[:, 2, :], sq4[:, 3, :],
                                op=ALU.add)
        nc.vector.tensor_tensor(sqs[0][:], sqs[0][:], sqs[1][:], op=ALU.add)
        nc.vector.tensor_tensor(sqs[1][:], sq4[:, 4, :], sq4[:, 5, :],
                                op=ALU.add)
        nc.vector.tensor_tensor(sqs[2][:], sq4[:, 6, :], sq4[:, 7, :],
                                op=ALU.add)
        nc.vector.tensor_tensor(sqs[1][:], sqs[1][:], sqs[2][:], op=ALU.add)
        nc.vector.tensor_tensor(sqs[0][:], sqs[0][:], sqs[1][:], op=ALU.add)
        nc.gpsimd.dma_start(
            out=ag_in[0:512, :].rearrange("(j p) n -> p j n", p=128),
            in_=seLh[:])

    # ---- phase B: query projection (chunk-interleaved m-group passes) ----
    with tc.tile_pool(name="psB", bufs=1, space="PSUM") as psB:
        for gi, (lo, hi) in enumerate(MGROUPS):
            pst = {}
            for ci, (c0, c1) in enumerate(CHUNKS):
                for i in range(hi - lo):
                    pst[(i, ci)] = psB.tile([128, c1 - c0], f32,
                                            tag=f"psB{i}c{ci}", name="psB")
            for g in range(KP):
                for ci, (c0, c1) in enumerate(CHUNKS):
                    qmov = qI[g // 4][:, g % 4, c0:c1, :] \
                        .rearrange("p n t -> p t n")
                    for i, m in enumerate(range(lo, hi)):
                        nc.tensor.matmul(
                            pst[(i, ci)][:],
                            w2s[g][:, :, m * 128:(m + 1) * 128],
                            qmov,
                            start=(g == 0), stop=(g == KP - 1),
                            perf_mode=DR,
                        )
            if gi == 0:
                # -s_sq = (-0.25) * colsum (2 se)^2, emitted here so the PE
                # never waits on the A-tail ACT/DVE chain
                with tc.tile_pool(name="psS", bufs=1, space="PSUM") as psSp:
                    psS = psSp.tile([1, SPC], f32, tag="ssq", name="ssq")
                    nc.tensor.matmul(psS[:], cst[:, 1:2], sqs[0][:],
                                     start=True, stop=True)
                    nc.vector.tensor_copy(nstA[0:1, 0:SPC], psS[:])
                nc.gpsimd.dma_start(
                    out=ag_in[512:514, :].bitcast(f16),
                    in_=nstA[:])
                if _MODE == "nocc":
                    for c in range(N_CORES):
                        nc.gpsimd.dma_start(out=ag_out[c], in_=ag_in[:])
                else:
                    nc.gpsimd.collective_compute(
                        "AllGather",
                        ALU.bypass,
                        replica_groups=[list(range(N_CORES))],
                        ins=[ag_in[:]],
                        outs=[ag_out[:]],
                    )
                # merge DMAs (gpsimd queue; they wait on the AG)
                for c in range(N_CORES):
                    nc.gpsimd.dma_start(
                        out=sePI[:, :, c * SPC:(c + 1) * SPC, :]
                        .rearrange("p j n t -> p j (n t)"),
                        in_=ag_out[c, 0:512, 0:2 * SPC]
                        .rearrange("(j p) n -> p j n", p=128))
                nc.gpsimd.dma_start(
                    out=nsr1h[:],
                    in_=ag_out[:, 512, 0:2 * SPC].bitcast(f16))
                nc.vector.tensor_copy(nsr1[:], nsr1h[:])
                nc.gpsimd.partition_broadcast(nsrB[:], nsr1[:])
            for ci, (c0, c1) in enumerate(CHUNKS):
                for i, m in enumerate(range(lo, hi)):
                    nc.scalar.activation(q8[:, m, c0:c1], pst[(i, ci)][:],
                                         AF.Relu, bias=bqc[:, m:m + 1],
                                         scale=1.0 / WSCALE)
                    nc.scalar.activation(sqt[:, m, c0:c1], q8[:, m, c0:c1],
                                         AF.Square)
        # q_sq rows: ones.T @ sqt (sum over dout partitions)
        with tc.tile_pool(name="psq", bufs=1, space="PSUM") as psq_pool:
            for c0, c1 in CHUNKS:
                psq = psq_pool.tile([1, c1 - c0], f32, tag=f"psq{c0}",
                                name="psq")
                for j in range(8):
                    nc.tensor.matmul(psq[:], cst[:, 0:1], sqt[:, j, c0:c1],
                                     start=(j == 0), stop=(j == 7))
                nc.vector.tensor_copy(qrow[0:1, c0:c1], psq[:])

    # q_sq row -> per-row-tile columns (partition scatter via DRAM bounce)
    nc.gpsimd.dma_start(out=qsd[:], in_=qrow[:])
    nc.gpsimd.dma_start(
        out=qsqc[:, 0:5],
        in_=qsd[0:1, 0:640].rearrange("o (t p) -> (o p) t", p=128))
    nc.gpsimd.dma_start(
        out=qsqc[0:88, 5:6],
        in_=qsd[0:1, 640:728].rearrange("o (t p) -> (o p) t", p=88))

    # ---- phase D: distance + (-s_sq add, per-class max) + mean ----
    with (
        tc.tile_pool(name="pd", bufs=4, space="PSUM") as pd_pool,
        tc.tile_pool(name="po", bufs=1, space="PSUM") as po_pool,
        tc.tile_pool(name="outs", bufs=1) as outs_pool,
    ):
        po = po_pool.tile([QPC, WAY], f32, tag="po", name="po")
        for mt in range(MT):
            mw = min(128, RPC - mt * 128)
            msl = slice(mt * 128, mt * 128 + mw)
            for ch in range(WAY):
                nsl = slice(ch * CLS, (ch + 1) * CLS)
                pd = pd_pool.tile([128, CLS], f32, tag="pd", name="pd")
                for jp in range(4):
                    nc.tensor.matmul(
                        pd[:mw, :],
                        q8[:, 2 * jp:2 * jp + 2, msl],
                        sePI[:, jp, nsl, :].rearrange("p n t -> p t n"),
                        start=(jp == 0), stop=(jp == 3),
                        perf_mode=DR,
                    )
                # M = 2 q.se - s_sq, then per-class max (both DVE)
                nc.vector.tensor_tensor(pd[:mw, :], pd[:mw, :],
                                        nsrB[:mw, nsl], op=ALU.add)
                nc.vector.tensor_reduce(
                    mins[mt][:mw, ch:ch + 1], pd[:mw, :],
                    axis=AX.X, op=ALU.max)
            # min d2 = q_sq - max M; d = sqrt(relu(.)) = sqrt(-min(M-q_sq,0))
            nc.vector.tensor_scalar(mins[mt][:mw, :], mins[mt][:mw, :],
                                    qsqc[:mw, mt:mt + 1], 0.0,
                                    ALU.subtract, ALU.min)
            nc.scalar.activation(mins[mt][:], mins[mt][:], AF.Sqrt,
                                 scale=-1.0)
            nc.tensor.matmul(po[:], mkt[mt][:], mins[mt][:],
                             start=(mt == 0), stop=(mt == MT - 1))

        out_s = outs_pool.tile([QPC, WAY], f32, tag="out_s", name="out_s")
        nc.vector.tensor_copy(out_s[:], po[:])
        nc.sync.dma_start(out=out[:], in_=out_s[:])

    dram_ctx.__exit__(None, None, None)
    persist_ctx.__exit__(None, None, None)


_NC_CACHE = {}


def _get_nc():
    if "nc" not in _NC_CACHE:
        _NC_CACHE["nc"] = _build_nc()
    return _NC_CACHE["nc"]


F8NP = ml_dtypes.float8_e4m3


def make_in_maps(support_set, support_labels, queries, clsW_w, clsW_b):
    support_set = np.asarray(support_set, dtype=np.float32)
    support_labels = np.asarray(support_labels)
    queries = np.asarray(queries, dtype=np.float32)
    clsW_w = np.asarray(clsW_w, dtype=np.float32)
    clsW_b = np.asarray(clsW_b, dtype=np.float32)

    # class-sort support rows so each class is a contiguous 280-column block
    perm = np.argsort(support_labels, kind="stable")
    S = support_set[perm].reshape(NSR, D_IN)
    STa = S.T.astype(F8NP)                            # [D_IN, NSR]
    # pair-interleaved blocked layout [6, 128, 4, n, 2]
    STi = np.ascontiguousarray(
        STa.reshape(6, 4, 2, 128, NSR).transpose(0, 3, 1, 4, 2))

    Qp = np.zeros((NQR, D_IN), np.float32)
    Qp[:N_Q * T] = queries.reshape(N_Q * T, D_IN)
    QTa = Qp.T.astype(F8NP)                           # [D_IN, NQR]
    QTi = np.ascontiguousarray(
        QTa.reshape(6, 4, 2, 128, NQR).transpose(0, 3, 1, 4, 2))

    WTa = (clsW_w.T * WSCALE).astype(F8NP)            # [D_IN, D_OUT]
    # per-k-pair tiles [24, 128, 2, D_OUT]
    W2b = np.ascontiguousarray(
        WTa.reshape(KP, 2, 128, D_OUT).transpose(0, 2, 1, 3))

    bqa = np.ascontiguousarray(clsW_b.reshape(8, 128).T)        # [128, 8]
    bsa = np.ascontiguousarray(bqa * 2.0)

    mmask = np.zeros((MT * 128, QPC), np.float32)
    r = np.arange(RPC)
    mmask[r, r // T] = -1.0 / T

    cstsa = np.zeros((128, 2), np.float16)
    cstsa[:, 0] = 1.0
    cstsa[:, 1] = -0.25

    in_maps = []
    for c in range(N_CORES):
        in_maps.append({
            "qT": np.ascontiguousarray(QTi[:, :, :, c * RPC:(c + 1) * RPC]),
            "w2": W2b,
            "sT": np.ascontiguousarray(
                STi[:, :, :, c * SPC:(c + 1) * SPC]),
            "bq": bqa,
            "bs": bsa,
            "mmask": mmask,
            "csts": cstsa,
        })
    return in_maps


def kernel(support_set, support_labels, queries, clsW_w, clsW_b):
    in_maps = make_in_maps(support_set, support_labels, queries, clsW_w,
                           clsW_b)
    nc = _get_nc()
    res = run_bass_kernel_spmd(nc, in_maps, list(range(N_CORES)))
    out = np.concatenate([res.results[c]["out"] for c in range(N_CORES)], 0)
    return np.ascontiguousarray(out[:N_Q]).astype(np.float32)
